# revision 1
# baseline (speedup 1.0000x reference)
"""Trainium2 kernel for nn_LJCH1_34548716929306 (ragged_sequence).

Strategy (pure data-parallel over batch, per sharding hint):
  - The dominant cost is the fc0 projection: concat([featContent,
    featDistort, motionFeat]) [16,2048,4864] @ fc0_w.T [4864,128].
    That is ~637MB of activations -> memory-regime. It runs on the 8
    NeuronCores, 2 samples per core, as scores.T = wT.T @ xT with
    feature-major (K-major) layout prepared host-side so the device
    streams contiguous tiles with zero on-chip transposes. bf16
    operands, fp32 PSUM accumulation.
  - The BiGRU over T=2048 (H=32) and the masked multi-scale softmax
    head are tiny (~0.1% of FLOPs) and sequential; they run in fp32
    numpy on host.
"""

import numpy as np
import ml_dtypes
from concurrent.futures import ThreadPoolExecutor
from contextlib import ExitStack

import concourse.bass as bass
import concourse.bacc as bacc
import concourse.tile as tile
from concourse import mybir
from concourse.bass_utils import run_bass_kernel_spmd
from concourse.kernels.tile_matmul import matmul_tile_kernel

B, T = 16, 2048
D_CONTENT, D_DISTORT, D_MOTION = 4096, 512, 256
D = D_CONTENT + D_DISTORT + D_MOTION  # 4864
RED, H = 128, 32
N_CORES = 8
BL = B // N_CORES  # 2 samples per core
TIME_INTERVAL = 2
NEG = -1e9

_compiled = None


def _build_nc():
    nc = bacc.Bacc(
        "TRN2",
        target_bir_lowering=False,
        debug=False,
        enable_asserts=False,
        num_devices=N_CORES,
    )
    xT = nc.dram_tensor("xT", [D, BL * T], mybir.dt.bfloat16, kind="ExternalInput")
    wT = nc.dram_tensor("wT", [D, RED], mybir.dt.bfloat16, kind="ExternalInput")
    sT = nc.dram_tensor("sT", [RED, BL * T], mybir.dt.float32, kind="ExternalOutput")
    with tile.TileContext(nc) as tc:
        matmul_tile_kernel(tc, wT.ap(), xT.ap(), sT.ap())
    nc.compile()
    return nc


def _get_compiled():
    global _compiled
    if _compiled is None:
        _compiled = _build_nc()
    return _compiled


_runner = None


def _get_runner():
    """Build the sharded PJRT executable once and reuse it across calls.

    run_bass_kernel_spmd's axon path re-traces and re-jits the shard_map
    wrapper on every invocation (fresh closures defeat the jit cache);
    caching it here removes multi-second per-call overhead.
    """
    global _runner
    if _runner is not None:
        return _runner
    import jax
    from jax.sharding import Mesh, PartitionSpec
    from jax.experimental.shard_map import shard_map
    from concourse import bass2jax
    from concourse import mybir as _mybir

    nc = _get_compiled()
    bass2jax.install_neuronx_cc_hook()

    partition_name = nc.partition_id_tensor.name if nc.partition_id_tensor else None
    in_names, out_names, out_avals = [], [], []
    for alloc in nc.m.functions[0].allocations:
        if not isinstance(alloc, _mybir.MemoryLocationSet):
            continue
        name = alloc.memorylocations[0].name
        if alloc.kind == "ExternalInput":
            if name != partition_name:
                in_names.append(name)
        elif alloc.kind == "ExternalOutput":
            out_names.append(name)
            out_avals.append(
                jax.core.ShapedArray(tuple(alloc.tensor_shape), _mybir.dt.np(alloc.dtype))
            )
    n_params = len(in_names)
    n_outs = len(out_avals)
    all_in_names = list(in_names) + list(out_names)
    if partition_name is not None:
        all_in_names.append(partition_name)
    donate = tuple(range(n_params, n_params + n_outs))

    def _body(*args):
        operands = list(args)
        if partition_name is not None:
            operands.append(bass2jax.partition_id_tensor())
        outs = bass2jax._bass_exec_p.bind(
            *operands,
            out_avals=tuple(out_avals),
            in_names=tuple(all_in_names),
            out_names=tuple(out_names),
            lowering_input_output_aliases=(),
            sim_require_finite=True,
            sim_require_nnan=True,
            nc=nc,
        )
        return tuple(outs)

    devices = jax.devices()[:N_CORES]
    mesh = Mesh(np.asarray(devices), ("core",))
    in_specs = (PartitionSpec("core"),) * (n_params + n_outs)
    out_specs = (PartitionSpec("core"),) * n_outs
    sharded = jax.jit(
        shard_map(_body, mesh=mesh, in_specs=in_specs, out_specs=out_specs,
                  check_rep=False),
        donate_argnums=donate,
        keep_unused=True,
    )

    def run(in_maps):
        concat_in = [
            np.concatenate([np.asarray(m[name]) for m in in_maps], axis=0)
            for name in in_names
        ]
        concat_zeros = [
            np.zeros((N_CORES * a.shape[0], *a.shape[1:]), a.dtype) for a in out_avals
        ]
        out_arrs = sharded(*concat_in, *concat_zeros)
        return [
            {
                name: np.asarray(out_arrs[i]).reshape(N_CORES, *out_avals[i].shape)[c]
                for i, name in enumerate(out_names)
            }
            for c in range(N_CORES)
        ]

    _runner = run
    # expose pieces for external timing/inspection (test harness use)
    global _sharded, _mesh, _in_names_g, _out_names_g, _out_avals_g
    _sharded, _mesh = sharded, mesh
    _in_names_g, _out_names_g, _out_avals_g = in_names, out_names, out_avals
    return _runner


def _run_device(in_maps):
    try:
        return _get_runner()(in_maps)
    except Exception:
        return run_bass_kernel_spmd(_get_compiled(), in_maps, list(range(N_CORES))).results


def _sigmoid(x):
    return 1.0 / (1.0 + np.exp(-x))


def _gru_dir(gi, wh, bh, reverse):
    # gi: [T, B, 3H] precomputed input gates; returns ys [T, B, H]
    Tn, Bn, _ = gi.shape
    whT = wh.T.copy()  # [H, 3H]
    h = np.zeros((Bn, H), np.float32)
    ys = np.empty((Tn, Bn, H), np.float32)
    order = range(Tn - 1, -1, -1) if reverse else range(Tn)
    for t in order:
        g = gi[t]
        gh = h @ whT + bh
        i_r, i_z, i_n = g[:, :H], g[:, H : 2 * H], g[:, 2 * H :]
        h_r, h_z, h_n = gh[:, :H], gh[:, H : 2 * H], gh[:, 2 * H :]
        r = _sigmoid(i_r + h_r)
        z = _sigmoid(i_z + h_z)
        n = np.tanh(i_n + r * h_n)
        h = (1.0 - z) * n + z * h
        ys[t] = h
    return ys


def _conv1d_same(x, w):
    # cross-correlation with zero 'same' padding; x [B,T], w [k]
    k = w.shape[0]
    p = k // 2
    xp = np.pad(x, ((0, 0), (p, p)))
    out = np.zeros_like(x)
    for j in range(k):
        out += w[j] * xp[:, j : j + x.shape[1]]
    return out


def _make_in_maps(inputs):
    fC = np.asarray(inputs["featContent"], np.float32)
    fD = np.asarray(inputs["featDistort"], np.float32)
    mF = np.asarray(inputs["motionFeat"], np.float32)
    fc0_w = np.asarray(inputs["fc0_w"], np.float32)
    # Host-side layout prep: per-core feature-major bf16 [D, BL*T].
    wT_np = np.ascontiguousarray(fc0_w.T).astype(ml_dtypes.bfloat16)

    def build(c):
        sl = slice(c * BL, (c + 1) * BL)
        xT = np.empty((D, BL * T), ml_dtypes.bfloat16)
        xT[:D_CONTENT] = fC[sl].reshape(BL * T, D_CONTENT).T
        xT[D_CONTENT : D_CONTENT + D_DISTORT] = fD[sl].reshape(BL * T, D_DISTORT).T
        xT[D_CONTENT + D_DISTORT :] = mF[sl].reshape(BL * T, D_MOTION).T
        return {"xT": xT, "wT": wT_np}

    with ThreadPoolExecutor(N_CORES) as ex:
        return list(ex.map(build, range(N_CORES)))


def kernel(**inputs):
    inputLength = np.asarray(inputs["inputLength"])
    fc0_b = np.asarray(inputs["fc0_b"], np.float32)

    in_maps = _make_in_maps(inputs)
    results = _run_device(in_maps)

    scores = np.empty((B, T, RED), np.float32)
    for c in range(N_CORES):
        sT = results[c]["sT"]  # [RED, BL*T]
        scores[c * BL : (c + 1) * BL] = (
            sT.T.reshape(BL, T, RED).astype(np.float32)
        )
    scores += fc0_b

    # BiGRU (fp32 host)
    x_tbd = scores.transpose(1, 0, 2)  # [T,B,RED]
    gi_f = x_tbd @ np.asarray(inputs["gru_wi_f"], np.float32).T + np.asarray(
        inputs["gru_bi_f"], np.float32
    )
    gi_b = x_tbd @ np.asarray(inputs["gru_wi_b"], np.float32).T + np.asarray(
        inputs["gru_bi_b"], np.float32
    )
    yf = _gru_dir(gi_f, np.asarray(inputs["gru_wh_f"], np.float32),
                  np.asarray(inputs["gru_bh_f"], np.float32), reverse=False)
    yb = _gru_dir(gi_b, np.asarray(inputs["gru_wh_b"], np.float32),
                  np.asarray(inputs["gru_bh_b"], np.float32), reverse=True)
    outputs = np.concatenate([yf, yb], -1).transpose(1, 0, 2)  # [B,T,2H]

    q_w = np.asarray(inputs["q_w"], np.float32)
    q_b = np.asarray(inputs["q_b"], np.float32)
    q = (outputs @ q_w.T + q_b)[..., 0]  # [B,T]

    lengths = inputLength.astype(np.int64) - 2 * (TIME_INTERVAL // 2) - 1
    mask = np.arange(T)[None, :] < lengths[:, None]
    qm = np.where(mask, q, 0.0).astype(np.float32)

    total = np.zeros((B,), np.float32)
    for wk in ("w1", "w2", "w3"):
        w = np.asarray(inputs[wk], np.float32)
        logits = np.where(mask, _conv1d_same(qm, w), NEG).astype(np.float32)
        m = logits.max(-1, keepdims=True)
        e = np.exp(logits - m)
        sm = e / e.sum(-1, keepdims=True)
        total = total + (sm * qm).sum(-1)
    return (total / 3.0)[:, None].astype(np.float32)



# revision 2
# speedup vs baseline: 460.3990x; 460.3990x over previous
"""Trainium2 kernel for nn_LJCH1_34548716929306 (ragged_sequence).

Strategy (pure data-parallel over batch, per sharding hint):
  - The dominant cost is the fc0 projection: concat([featContent,
    featDistort, motionFeat]) [16,2048,4864] @ fc0_w.T [4864,128].
    ~637MB fp32 of activations -> memory-regime. Runs on the 8
    NeuronCores, 2 samples per core, as sT = W^T-packed @ xT with
    feature-major (K-major) bf16 layout prepared host-side.
  - Custom Bass/Tile kernel per core:
      * fc0 weights packed [128(K-part), 38*128] resident in SBUF
      * x streamed in 4 super-chunks of [128, 38, 1024] bf16 (~10MB),
        double-buffered; one dma_start per super-chunk (2KB lines)
      * 38-step PSUM fp32 accumulation per 512-col chunk, 8 chunks
      * result copied PSUM->SBUF (DVE) and DMA'd out as fp32
  - `reps` builds the same kernel with the whole body inside a
    hardware For_i loop (identical addresses per iteration; weights
    hoisted out). Used by the test harness to measure steady-state
    per-iteration HW time as a slope, cancelling the axon-tunnel RTT.
  - The BiGRU over T=2048 (H=32) and the masked multi-scale softmax
    head are tiny (~0.1% of FLOPs) and sequential; they run in fp32
    numpy on host.
"""

import numpy as np
import ml_dtypes
from concurrent.futures import ThreadPoolExecutor

import concourse.bass as bass
import concourse.bacc as bacc
import concourse.tile as tile
from concourse import mybir
from concourse.bass import ds, ts
from concourse.bass_utils import run_bass_kernel_spmd

B, T = 16, 2048
D_CONTENT, D_DISTORT, D_MOTION = 4096, 512, 256
D = D_CONTENT + D_DISTORT + D_MOTION  # 4864
RED, H = 128, 32
N_CORES = 8
BL = B // N_CORES  # 2 samples per core
NCOL = BL * T  # 4096 columns per core
KT = D // 128  # 38 K-tiles
TIME_INTERVAL = 2
NEG = -1e9

_compiled = {}  # reps -> Bacc


def _build_nc(reps=1):
    nc = bacc.Bacc(
        "TRN2",
        target_bir_lowering=False,
        debug=False,
        enable_asserts=False,
        num_devices=N_CORES,
    )
    x3 = nc.dram_tensor("x3", [KT, 128, NCOL], mybir.dt.bfloat16, kind="ExternalInput")
    w = nc.dram_tensor("w", [128, D], mybir.dt.bfloat16, kind="ExternalInput")
    sT = nc.dram_tensor("sT", [RED, NCOL], mybir.dt.float32, kind="ExternalOutput")

    CH = 1024  # super-chunk columns (bf16 -> 2KB per-partition DMA lines)
    n_ch = NCOL // CH  # 4

    with tile.TileContext(nc) as tc:
        with tc.tile_pool(name="wp", bufs=1) as wp, \
             tc.tile_pool(name="xp", bufs=2) as xp, \
             tc.tile_pool(name="op", bufs=3) as op, \
             tc.tile_pool(name="pp", bufs=4, space="PSUM") as pp:
            w_sb = wp.tile([128, D], mybir.dt.bfloat16)
            nc.sync.dma_start(w_sb[:], w.ap())

            def body():
                for j in range(n_ch):
                    xt = xp.tile([128, KT, CH], mybir.dt.bfloat16, tag="x")
                    nc.sync.dma_start(
                        xt[:],
                        x3.ap()[:, :, ds(j * CH, CH)].rearrange("k p c -> p k c"),
                    )
                    for h in range(CH // 512):
                        ps = pp.tile([128, 512], mybir.dt.float32, tag="ps")
                        for k in range(KT):
                            nc.tensor.matmul(
                                ps[:],
                                w_sb[:, ts(k, 128)],
                                xt[:, k : k + 1, ds(h * 512, 512)],
                                start=(k == 0),
                                stop=(k == KT - 1),
                            )
                        ot = op.tile([128, 512], mybir.dt.float32, tag="o")
                        nc.vector.tensor_copy(ot[:], ps[:])
                        nc.sync.dma_start(
                            sT.ap()[:, ds(j * CH + h * 512, 512)], ot[:]
                        )

            if reps == 1:
                body()
            else:
                with tc.For_i(0, reps, hint_engines=(mybir.EngineType.PE,)):
                    body()
    nc.compile()
    return nc


def _get_compiled(reps=1):
    if reps not in _compiled:
        _compiled[reps] = _build_nc(reps)
    return _compiled[reps]


_runners = {}  # reps -> dict(run=..., sharded=..., mesh=..., names...)


def _make_runner(reps=1):
    """Build the sharded PJRT executable once and reuse it across calls.

    Uses bass2jax fast-dispatch (no effects -> C++ dispatch path) and no
    donation so the same device-resident buffers can be re-executed.
    """
    import jax
    from jax.sharding import Mesh, PartitionSpec
    from jax.experimental.shard_map import shard_map
    from concourse import bass2jax
    from concourse import mybir as _mybir

    nc = _get_compiled(reps)
    bass2jax.install_neuronx_cc_hook()

    partition_name = nc.partition_id_tensor.name if nc.partition_id_tensor else None
    in_names, out_names, out_avals = [], [], []
    for alloc in nc.m.functions[0].allocations:
        if not isinstance(alloc, _mybir.MemoryLocationSet):
            continue
        name = alloc.memorylocations[0].name
        if alloc.kind == "ExternalInput":
            if name != partition_name:
                in_names.append(name)
        elif alloc.kind == "ExternalOutput":
            out_names.append(name)
            out_avals.append(
                jax.core.ShapedArray(tuple(alloc.tensor_shape), _mybir.dt.np(alloc.dtype))
            )
    all_in_names = list(in_names) + list(out_names)
    if partition_name is not None:
        all_in_names.append(partition_name)

    def _body(*args):
        operands = list(args)
        if partition_name is not None:
            operands.append(bass2jax.partition_id_tensor())
        outs = bass2jax._bass_exec_p.bind(
            *operands,
            out_avals=tuple(out_avals),
            in_names=tuple(all_in_names),
            out_names=tuple(out_names),
            lowering_input_output_aliases=(),
            sim_require_finite=True,
            sim_require_nnan=True,
            nc=nc,
        )
        return tuple(outs)

    devices = jax.devices()[:N_CORES]
    mesh = Mesh(np.asarray(devices), ("core",))
    n_params, n_outs = len(in_names), len(out_avals)
    in_specs = (PartitionSpec("core"),) * (n_params + n_outs)
    out_specs = (PartitionSpec("core"),) * n_outs

    in_shapes = {"x3": (N_CORES * KT, 128, NCOL), "w": (N_CORES * 128, D)}
    avals = [jax.ShapeDtypeStruct(in_shapes[n], ml_dtypes.bfloat16) for n in in_names]
    avals += [
        jax.ShapeDtypeStruct((N_CORES * a.shape[0], *a.shape[1:]), a.dtype)
        for a in out_avals
    ]

    def _jit():
        return jax.jit(
            shard_map(_body, mesh=mesh, in_specs=in_specs, out_specs=out_specs,
                      check_rep=False),
            keep_unused=True,
        )

    try:
        sharded = bass2jax.fast_dispatch_compile(
            lambda: _jit().lower(*avals).compile()
        )
    except Exception:
        sharded = _jit()

    def run(in_maps):
        from jax.sharding import NamedSharding
        sh = NamedSharding(mesh, PartitionSpec("core"))
        concat_in = [
            np.concatenate([np.asarray(m[name]) for m in in_maps], axis=0)
            for name in in_names
        ]
        concat_zeros = [
            np.zeros((N_CORES * a.shape[0], *a.shape[1:]), a.dtype) for a in out_avals
        ]
        out_arrs = sharded(*[jax.device_put(a, sh) for a in concat_in + concat_zeros])
        return [
            {
                name: np.asarray(out_arrs[i]).reshape(N_CORES, *out_avals[i].shape)[c]
                for i, name in enumerate(out_names)
            }
            for c in range(N_CORES)
        ]

    return dict(
        run=run, sharded=sharded, mesh=mesh,
        in_names=in_names, out_names=out_names, out_avals=out_avals,
    )


def _get_runner(reps=1):
    if reps not in _runners:
        _runners[reps] = _make_runner(reps)
    return _runners[reps]


def _run_device(in_maps):
    try:
        return _get_runner(1)["run"](in_maps)
    except Exception:
        return run_bass_kernel_spmd(_get_compiled(1), in_maps, list(range(N_CORES))).results


def _sigmoid(x):
    return 1.0 / (1.0 + np.exp(-x))


def _gru_dir(gi, wh, bh, reverse):
    # gi: [T, B, 3H] precomputed input gates; returns ys [T, B, H]
    Tn, Bn, _ = gi.shape
    whT = wh.T.copy()  # [H, 3H]
    h = np.zeros((Bn, H), np.float32)
    ys = np.empty((Tn, Bn, H), np.float32)
    order = range(Tn - 1, -1, -1) if reverse else range(Tn)
    for t in order:
        g = gi[t]
        gh = h @ whT + bh
        i_r, i_z, i_n = g[:, :H], g[:, H : 2 * H], g[:, 2 * H :]
        h_r, h_z, h_n = gh[:, :H], gh[:, H : 2 * H], gh[:, 2 * H :]
        r = _sigmoid(i_r + h_r)
        z = _sigmoid(i_z + h_z)
        n = np.tanh(i_n + r * h_n)
        h = (1.0 - z) * n + z * h
        ys[t] = h
    return ys


def _conv1d_same(x, w):
    # cross-correlation with zero 'same' padding; x [B,T], w [k]
    k = w.shape[0]
    p = k // 2
    xp = np.pad(x, ((0, 0), (p, p)))
    out = np.zeros_like(x)
    for j in range(k):
        out += w[j] * xp[:, j : j + x.shape[1]]
    return out


def _pack_weights(fc0_w):
    # w_packed[p, k*128+m] = fc0_w[m, k*128+p]  (lhsT tiles side by side)
    return (
        np.ascontiguousarray(
            fc0_w.T.reshape(KT, 128, RED).transpose(1, 0, 2).reshape(128, D)
        ).astype(ml_dtypes.bfloat16)
    )


def _make_in_maps(inputs):
    fC = np.asarray(inputs["featContent"], np.float32)
    fD = np.asarray(inputs["featDistort"], np.float32)
    mF = np.asarray(inputs["motionFeat"], np.float32)
    fc0_w = np.asarray(inputs["fc0_w"], np.float32)
    w_np = _pack_weights(fc0_w)

    def build(c):
        sl = slice(c * BL, (c + 1) * BL)
        xT = np.empty((D, NCOL), ml_dtypes.bfloat16)
        xT[:D_CONTENT] = fC[sl].reshape(NCOL, D_CONTENT).T
        xT[D_CONTENT : D_CONTENT + D_DISTORT] = fD[sl].reshape(NCOL, D_DISTORT).T
        xT[D_CONTENT + D_DISTORT :] = mF[sl].reshape(NCOL, D_MOTION).T
        return {"x3": xT.reshape(KT, 128, NCOL), "w": w_np}

    with ThreadPoolExecutor(N_CORES) as ex:
        return list(ex.map(build, range(N_CORES)))


def kernel(**inputs):
    inputLength = np.asarray(inputs["inputLength"])
    fc0_b = np.asarray(inputs["fc0_b"], np.float32)

    in_maps = _make_in_maps(inputs)
    results = _run_device(in_maps)

    scores = np.empty((B, T, RED), np.float32)
    for c in range(N_CORES):
        sT = results[c]["sT"]  # [RED, BL*T]
        scores[c * BL : (c + 1) * BL] = (
            sT.T.reshape(BL, T, RED).astype(np.float32)
        )
    scores += fc0_b

    # BiGRU (fp32 host)
    x_tbd = scores.transpose(1, 0, 2)  # [T,B,RED]
    gi_f = x_tbd @ np.asarray(inputs["gru_wi_f"], np.float32).T + np.asarray(
        inputs["gru_bi_f"], np.float32
    )
    gi_b = x_tbd @ np.asarray(inputs["gru_wi_b"], np.float32).T + np.asarray(
        inputs["gru_bi_b"], np.float32
    )
    yf = _gru_dir(gi_f, np.asarray(inputs["gru_wh_f"], np.float32),
                  np.asarray(inputs["gru_bh_f"], np.float32), reverse=False)
    yb = _gru_dir(gi_b, np.asarray(inputs["gru_wh_b"], np.float32),
                  np.asarray(inputs["gru_bh_b"], np.float32), reverse=True)
    outputs = np.concatenate([yf, yb], -1).transpose(1, 0, 2)  # [B,T,2H]

    q_w = np.asarray(inputs["q_w"], np.float32)
    q_b = np.asarray(inputs["q_b"], np.float32)
    q = (outputs @ q_w.T + q_b)[..., 0]  # [B,T]

    lengths = inputLength.astype(np.int64) - 2 * (TIME_INTERVAL // 2) - 1
    mask = np.arange(T)[None, :] < lengths[:, None]
    qm = np.where(mask, q, 0.0).astype(np.float32)

    total = np.zeros((B,), np.float32)
    for wk in ("w1", "w2", "w3"):
        w = np.asarray(inputs[wk], np.float32)
        logits = np.where(mask, _conv1d_same(qm, w), NEG).astype(np.float32)
        m = logits.max(-1, keepdims=True)
        e = np.exp(logits - m)
        sm = e / e.sum(-1, keepdims=True)
        total = total + (sm * qm).sum(-1)
    return (total / 3.0)[:, None].astype(np.float32)
